# revision 23
# baseline (speedup 1.0000x reference)
"""Causal self-attention with ALiBi on 8 trn2 cores.

Sharding: data-parallel over batch (2) x tensor-parallel over head groups (4).
Core c handles batch b = c // 4, head group g = c % 4 (heads 4g..4g+3).

All matmul paths run in bf16 (1 cycle/row on PE at any tile width).  The
ALiBi bias is folded into the score matmul via 3 augmented rows
(k_aug = [j_hi; j_lo; 1], q_aug = [slope; slope; -slope*i] with
j = j_hi + j_lo, both bf16-exact), so scores are exact up to a per-query
constant shift that softmax cancels.  The softmax denominator comes from a
ones column appended to V (row 64 of the [65, 512] AV accumulator), is
reciprocated in one custom-DVE op straight out of PSUM, and is broadcast
across partitions with a K=1 PE matmul (no DRAM round-trip).

ALiBi makes attention local: keys with slope*dist > 24 contribute < e^-20
relative mass, so score/AV chunks entirely beyond the per-head window
(384/(h+1) keys for head slot h, using group 0's slopes as the safe upper
bound for every group) are skipped.

Engine budget: PE does all matmuls; ACT does exp (+ half the output
evacuations); DVE does reciprocal + normalize (+ the other half); Pool does
q/k/v PSUM evacuations; inputs arrive as one large contiguous DMA per
tensor (host pre-tiles into the SBUF layout).  The program interleaves
next-block projections and lagged output projections between attention
heads so the PE stream never waits on the exp chain.
"""

import sys

sys.path.insert(0, "/opt/trn_rl_repo")

import numpy as np

import concourse.bacc as bacc
import concourse.mybir as mybir
import concourse.tile as tile
from concourse.bass import ds, ts
from concourse.bass_utils import run_bass_kernel_spmd

B, T, D, H, DH = 2, 2048, 1024, 16, 64
G = 4            # head groups (tensor-parallel)
HPC = H // G     # heads per core
DG = D // G      # model dims per core (256)
P = 128
N_CORES = 8
NEG = -1.0e30
CUT = 24.0       # ALiBi cutoff in score units; dropped mass < T*e^(4-CUT)

F32 = mybir.dt.float32
F32R = mybir.dt.float32r
BF16 = mybir.dt.bfloat16
ADD = mybir.AluOpType.add
MULT = mybir.AluOpType.mult
EXP = mybir.ActivationFunctionType.Exp
COPYF = mybir.ActivationFunctionType.Copy

TRACE = False
LAST_RESULTS = None

_cache = {}

# windows for local head slot h: worst case over groups is group 0
# (slope = (h+1)/16), so W = CUT*16/(h+1) keys is safe for every core.
WIN = [CUT * 16.0 / (h + 1) for h in range(HPC)]


def _jc_min(h, qb):
    o = qb * 512
    return max(0, int(np.ceil((o - WIN[h] - 127) / 128.0)))


def _build(with_bias: bool):
    nc = bacc.Bacc("TRN2", target_bir_lowering=False, debug=False)

    x_d = nc.dram_tensor("xTb", [4, P, 8, 512], BF16, kind="ExternalInput").ap()
    wq_d = nc.dram_tensor("wqT", [P, 8, DG], BF16, kind="ExternalInput").ap()
    wk_d = nc.dram_tensor("wkT", [P, 8, DG], BF16, kind="ExternalInput").ap()
    wv_d = nc.dram_tensor("wvT", [P, 8, HPC, DH], BF16, kind="ExternalInput").ap()
    wo_d = nc.dram_tensor("woT", [P, 2, D], BF16, kind="ExternalInput").ap()
    qaug_d = nc.dram_tensor("qaug", [3, HPC, T], BF16, kind="ExternalInput").ap()
    kaug_d = nc.dram_tensor("kaug", [3, HPC, T], BF16, kind="ExternalInput").ap()
    ident_d = nc.dram_tensor("ident", [P, P], BF16, kind="ExternalInput").ap()
    maskst_d = nc.dram_tensor("maskst", [P, P], BF16, kind="ExternalInput").ap()
    if with_bias:
        bq_d = nc.dram_tensor("bq2", [P, 2], F32, kind="ExternalInput").ap()
        bk_d = nc.dram_tensor("bk2", [P, 2], F32, kind="ExternalInput").ap()
        bvo_d = nc.dram_tensor("bvo", [P, HPC, DH], F32, kind="ExternalInput").ap()
    out_d = nc.dram_tensor("outT", [D, T], BF16, kind="ExternalOutput").ap()

    with tile.TileContext(nc) as tc:
        with (
            tc.tile_pool(name="big", bufs=1) as big,
            tc.tile_pool(name="stage", bufs=3) as stage,
            tc.tile_pool(name="expp", bufs=5) as expp,
            tc.tile_pool(name="dnp", bufs=2) as dnp,
            tc.tile_pool(name="mm", bufs=2, space="PSUM") as mmp,
            tc.tile_pool(name="pss", bufs=4, space="PSUM") as pssp,
            tc.tile_pool(name="psy", bufs=2, space="PSUM") as psyp,
        ):
            # ---- persistent tiles
            wv = big.tile([P, 8, HPC, DH], BF16, tag="wv", name="wv")
            wq = big.tile([P, 8, DG], BF16, tag="wq", name="wq")
            wk = big.tile([P, 8, DG], BF16, tag="wk", name="wk")
            wo = big.tile([P, 2, D], BF16, tag="wo", name="wo")
            qa = big.tile([67, HPC, T], BF16, tag="qa", name="qa")
            ka = big.tile([67, HPC, T], BF16, tag="ka", name="ka")
            va = big.tile([P, 16, HPC, 65], BF16, tag="va", name="va")
            yt = [big.tile([P, T], BF16, tag=f"yt{m}", name=f"yt{m}") for m in range(2)]
            ident_sb = big.tile([P, P], BF16, tag="ident")
            maskst_sb = big.tile([P, P], BF16, tag="maskst")
            ones64 = big.tile([1, 64], F32, tag="ones64")
            xt = [big.tile([P, 8, 512], BF16, tag=f"xt{t}", name=f"xt{t}") for t in range(4)]
            if with_bias:
                bq2 = big.tile([P, 2], F32, tag="bq2")
                bk2 = big.tile([P, 2], F32, tag="bk2")
                bvo = big.tile([P, HPC, DH], F32, tag="bvo")

            # ---- input DMAs: few, large, ordered so the first t-block and
            # the weights land before the PE needs them.
            nc.sync.dma_start(out=ident_sb[:], in_=ident_d[:])
            nc.sync.dma_start(out=maskst_sb[:], in_=maskst_d[:])
            nc.sync.dma_start(out=xt[0][:, 0:4, :], in_=x_d[0][:, 0:4, :])
            nc.sync.dma_start(out=wk[:], in_=wk_d[:])
            nc.sync.dma_start(out=ka[64:67, :, :], in_=kaug_d[:])
            nc.sync.dma_start(out=qa[64:67, :, :], in_=qaug_d[:])
            nc.sync.dma_start(out=wo[:], in_=wo_d[:])
            for tq in range(1, 4):
                nc.sync.dma_start(out=xt[tq][:], in_=x_d[tq])

            nc.scalar.dma_start(out=wv[:], in_=wv_d[:])
            nc.scalar.dma_start(out=xt[0][:, 4:8, :], in_=x_d[0][:, 4:8, :])
            nc.scalar.dma_start(out=wq[:], in_=wq_d[:])
            if with_bias:
                nc.scalar.dma_start(out=bq2[:], in_=bq_d[:])
                nc.scalar.dma_start(out=bk2[:], in_=bk_d[:])
                nc.scalar.dma_start(out=bvo[:], in_=bvo_d[:])

            # preload the Exp table while waiting on weights
            nc.vector.memset(ones64[0:1, :], 1.0)
            dummy = dnp.tile([1, 512], F32, tag="dn", name="dummy")
            nc.scalar.activation(out=dummy[0:1, 0:64], in_=ones64[0:1, :], func=EXP)

            # ones columns for the in-matmul softmax denominator
            for ch in range(16):
                nc.gpsimd.memset(va[:, ch, :, 64:65], 1.0)

            # ---- PE warmup: ramp the p-state while the x/weight DMAs land
            for i in range(24):
                wps = pssp.tile([P, 512], F32, tag="pss", name=f"warm{i}")
                nc.tensor.matmul(out=wps[:, 0:P], lhsT=ident_sb[:], rhs=maskst_sb[:], start=True, stop=True)

            def emit_vproj(tq, ch):
                lc = (ch % 4) * P
                pv = mmp.tile([P, HPC, DH], F32, tag="mm", name=f"pv{ch}")
                for kc in range(8):
                    nc.tensor.matmul(
                        out=pv[:, :, :],
                        lhsT=xt[tq][:, kc, lc : lc + P],
                        rhs=wv[:, kc, :, :],
                        start=(kc == 0),
                        stop=(kc == 7),
                    )
                if with_bias:
                    nc.vector.tensor_tensor(
                        out=va[:, ch, :, 0:64], in0=pv[:, :, :], in1=bvo[:, :, :], op=ADD
                    )
                else:
                    nc.scalar.activation(out=va[:, ch, :, 0:64], in_=pv[:, :, :], func=COPYF)

            def emit_qkproj(tq, piece):
                wt, dst = [(wq, qa), (wk, ka)][piece // 2]
                mc = piece % 2
                pq = mmp.tile([P, 512], F32, tag="mm", name=f"pq{tq}_{piece}")
                for kc in range(8):
                    nc.tensor.matmul(
                        out=pq[:],
                        lhsT=wt[:, kc, ts(mc, P)],
                        rhs=xt[tq][:, kc, :],
                        start=(kc == 0),
                        stop=(kc == 7),
                    )
                bt = (bq2 if piece // 2 == 0 else bk2) if with_bias else None
                for half in range(2):
                    h = 2 * mc + half
                    src = pq[half * 64 : half * 64 + 64, :]
                    if with_bias:
                        nc.vector.tensor_scalar(
                            out=dst[0:64, h, ts(tq, 512)],
                            in0=src,
                            scalar1=bt[half * 64 : half * 64 + 64, mc : mc + 1],
                            scalar2=None,
                            op0=ADD,
                        )
                    else:
                        nc.vector.tensor_copy(out=dst[0:64, h, ts(tq, 512)], in_=src)

            pend_a = [None]  # (qb, h, py) awaiting recip+broadcast
            pend_b = [None]  # (qb, h, py, pbs) awaiting the normalize multiply

            def flush_recip():
                pqb, ph, ppy = pend_a[0]
                dnin = dnp.tile([1, 512], F32, tag="dnin", name=f"dnin{pqb}_{ph}")
                nc.vector.tensor_copy(out=dnin[0:1, :], in_=ppy[64:65, :])
                dn2 = dnp.tile([1, 512], F32, tag="dn", name=f"dn{pqb}_{ph}")
                nc.vector.reciprocal_approx_fast(out=dn2[0:1, :], in_=dnin[0:1, :])
                pbs = stage.tile([64, 512], F32, tag="pbs", name=f"pbs{pqb}_{ph}")
                nc.gpsimd.partition_broadcast(pbs[:, :], dn2[0:1, :])
                pend_a[0] = None
                pend_b[0] = (pqb, ph, ppy, pbs)

            def flush_mult():
                pqb, ph, ppy, pbs = pend_b[0]
                nc.vector.tensor_tensor(
                    out=yt[ph // 2][(ph % 2) * 64 : (ph % 2) * 64 + 64, ts(pqb, 512)],
                    in0=ppy[0:64, :],
                    in1=pbs[:, :],
                    op=MULT,
                )
                pend_b[0] = None

            def flush_all():
                if pend_a[0] is not None:
                    flush_recip()
                if pend_b[0] is not None:
                    flush_mult()

            def emit_head(qb, h, fillers=()):
                fillers = list(fillers)
                o = qb * 512
                jcs = list(range(_jc_min(h, qb), 4 * qb + 4))
                py = psyp.tile([65, 512], F32, tag="psy", name=f"py{qb}_{h}")
                q = []
                first = [True]

                def av(item, stop):
                    jc, lo, ex = item
                    nc.tensor.matmul(
                        out=py[:, lo:512],
                        lhsT=va[:, jc, h, :],
                        rhs=ex[:, lo:512],
                        start=first[0],
                        stop=stop,
                    )
                    first[0] = False

                for idx, jc in enumerate(jcs):
                    r = jc * P - o
                    lo = max(r, 0)
                    ps = pssp.tile([P, 512], F32, tag="pss", name=f"ps{qb}_{h}_{jc}")
                    if r < 0:
                        nc.tensor.matmul(
                            out=ps[:],
                            lhsT=ka[:, h, ts(jc, P)],
                            rhs=qa[:, h, ds(o, 512)],
                            start=True,
                            stop=True,
                        )
                    else:
                        nc.tensor.matmul(
                            out=ps[:, lo:512],
                            lhsT=ka[:, h, ts(jc, P)],
                            rhs=qa[:, h, ds(o + lo, 512 - lo)],
                            start=True,
                            stop=False,
                        )
                        # causal stair: ps[:, lo:lo+128] += I.T @ maskst
                        nc.tensor.matmul(
                            out=ps[:, lo : lo + P],
                            lhsT=ident_sb[:],
                            rhs=maskst_sb[:],
                            start=False,
                            stop=True,
                        )
                    ex = expp.tile([P, 512], BF16, tag="ex", name=f"ex{qb}_{h}_{jc}")
                    nc.scalar.activation(out=ex[:, lo:512], in_=ps[:, lo:512], func=EXP)
                    q.append((jc, lo, ex))
                    if idx == 1 and pend_a[0] is not None:
                        flush_recip()
                    if idx == 3 and pend_b[0] is not None:
                        flush_mult()
                    if idx in (4, 6) and fillers:
                        fillers.pop(0)()
                    if len(q) > 3:
                        av(q.pop(0), False)
                while q:
                    av(q.pop(0), len(q) == 1)
                for f in fillers:
                    f()
                flush_all()  # safety for short heads
                pend_a[0] = (qb, h, py)

            def emit_outproj(qb, ec, act_ok):
                po = mmp.tile([P, 512], F32, tag="mm", name=f"po{qb}_{ec}")
                for k2 in range(2):
                    nc.tensor.matmul(
                        out=po[:],
                        lhsT=wo[:, k2, ts(ec, P)],
                        rhs=yt[k2][:, ts(qb, 512)],
                        start=(k2 == 0),
                        stop=(k2 == 1),
                    )
                ob = stage.tile([P, 512], BF16, tag="ob", name=f"ob{qb}_{ec}")
                if act_ok and ec % 2 == 0:
                    nc.scalar.activation(out=ob[:], in_=po[:], func=COPYF)
                else:
                    nc.vector.tensor_copy(out=ob[:], in_=po[:])
                nc.sync.dma_start(out=out_d[ts(ec, P), ts(qb, 512)], in_=ob[:])

            # ---- schedule
            for j in range(4):
                emit_vproj(0, j)
                emit_qkproj(0, j)

            def op_filler(qb, ec, act_ok):
                return lambda: emit_outproj(qb, ec, act_ok)

            for qb in range(4):
                for h in range(HPC):
                    fillers = []
                    if qb > 0:
                        act_ok = qb < 3
                        fillers = [op_filler(qb - 1, ec, act_ok) for ec in (2 * h, 2 * h + 1)]
                    emit_head(qb, h, fillers)
                    if qb < 3:
                        emit_vproj(qb + 1, (qb + 1) * 4 + h)
                        emit_qkproj(qb + 1, h)
            flush_all()
            for ec in range(8):
                emit_outproj(3, ec, act_ok=True)

    nc.compile()
    return nc


def _get_nc(with_bias: bool):
    if with_bias not in _cache:
        _cache[with_bias] = _build(with_bias)
    return _cache[with_bias]


def kernel(x, freqs_cis, Wq, bq, Wkv, bkv, Wo, bo, **_unused):
    import ml_dtypes

    bf = ml_dtypes.bfloat16
    x = np.asarray(x, np.float32)
    Wq = np.asarray(Wq, np.float32)
    bq = np.asarray(bq, np.float32)
    Wkv = np.asarray(Wkv, np.float32)
    bkv = np.asarray(bkv, np.float32)
    Wo = np.asarray(Wo, np.float32)
    bo = np.asarray(bo, np.float32)

    with_bias = bool(np.any(bq) or np.any(bkv))
    nc = _get_nc(with_bias)

    scale = 1.0 / np.sqrt(DH)
    iota = np.arange(T, dtype=np.float32)
    j_hi = np.floor(iota / 256.0) * 256.0        # bf16-exact
    j_lo = iota - j_hi                           # 0..255, bf16-exact
    kaug1 = np.stack([j_hi, j_lo, np.ones(T, np.float32)])          # [3,T]
    kaug = np.broadcast_to(kaug1[:, None, :], (3, HPC, T)).astype(bf)

    mm = np.arange(P, dtype=np.float32)
    maskst = np.where(mm[None, :] < mm[:, None], NEG, 0.0).astype(bf)
    ident = np.eye(P, dtype=np.float32).astype(bf)

    # x pre-tiled to the SBUF layout: [tq, partition, kc, t] so each t-block
    # is one contiguous DMA.
    xTb = [
        np.ascontiguousarray(
            x[b].T.reshape(8, P, 4, 512).transpose(2, 1, 0, 3)
        ).astype(bf)
        for b in range(B)
    ]

    in_maps = []
    for c in range(N_CORES):
        b, g = divmod(c, G)
        rows = slice(g * DG, (g + 1) * DG)
        # [D, DG] -> [P, 8, DG] (partition-major with kc on the free dim)
        wqT = np.ascontiguousarray(
            (Wq[rows] * scale).T.reshape(8, P, DG).transpose(1, 0, 2)
        ).astype(bf)
        wkT = np.ascontiguousarray(
            Wkv[0:D][rows].T.reshape(8, P, DG).transpose(1, 0, 2)
        ).astype(bf)
        wvT = np.ascontiguousarray(
            Wkv[D : 2 * D][rows].T.reshape(8, P, HPC, DH).transpose(1, 0, 2, 3)
        ).astype(bf)
        woT = np.ascontiguousarray(
            Wo[:, rows].T.reshape(2, P, D).transpose(1, 0, 2)
        ).astype(bf)
        qaug = np.zeros((3, HPC, T), np.float32)
        for h in range(HPC):
            slope = (g * HPC + h + 1) / H
            qaug[0, h, :] = slope
            qaug[1, h, :] = slope
            qaug[2, h, :] = -slope * iota
        m = {
            "xTb": xTb[b],
            "wqT": wqT,
            "wkT": wkT,
            "wvT": wvT,
            "woT": woT,
            "qaug": qaug.astype(bf),
            "kaug": kaug,
            "ident": ident,
            "maskst": maskst,
        }
        if with_bias:
            m["bq2"] = np.ascontiguousarray((bq[rows] * scale).reshape(2, P).T)
            m["bk2"] = np.ascontiguousarray(bkv[0:D][rows].reshape(2, P).T)
            m["bvo"] = np.broadcast_to(
                bkv[D : 2 * D][rows].reshape(HPC, DH)[None], (P, HPC, DH)
            ).astype(np.float32).copy()
        in_maps.append(m)

    res = run_bass_kernel_spmd(nc, in_maps, list(range(N_CORES)), trace=TRACE)
    global LAST_RESULTS
    LAST_RESULTS = res

    out = np.empty((B, T, D), np.float32)
    for b in range(B):
        acc = res.results[b * G]["outT"].astype(np.float32)
        for g in range(1, G):
            acc += res.results[b * G + g]["outT"].astype(np.float32)
        out[b] = acc.T + bo[None, :]
    return out


# revision 29
# speedup vs baseline: 1.0088x; 1.0088x over previous
"""Causal self-attention with ALiBi on 8 trn2 cores.

Sharding: data-parallel over batch (2) x tensor-parallel over head groups (4).
Core c handles batch b = c // 4, head group g = c % 4 (heads 4g..4g+3).

All matmul paths run in bf16 (1 cycle/row on PE at any tile width).  The
ALiBi bias is folded into the score matmul via 3 augmented rows
(k_aug = [j_hi; j_lo; 1], q_aug = [slope; slope; -slope*i] with
j = j_hi + j_lo, both bf16-exact), so scores are exact up to a per-query
constant shift that softmax cancels.  The softmax denominator comes from a
ones column appended to V (row 64 of the [65, 512] AV accumulator), is
reciprocated in one custom-DVE op straight out of PSUM, and is broadcast
across partitions with a K=1 PE matmul (no DRAM round-trip).

ALiBi makes attention local: keys with slope*dist > 24 contribute < e^-20
relative mass, so score/AV chunks entirely beyond the per-head window
(384/(h+1) keys for head slot h, using group 0's slopes as the safe upper
bound for every group) are skipped.

Engine budget: PE does all matmuls; ACT does exp (+ half the output
evacuations); DVE does reciprocal + normalize (+ the other half); Pool does
q/k/v PSUM evacuations; inputs arrive as one large contiguous DMA per
tensor (host pre-tiles into the SBUF layout).  The program interleaves
next-block projections and lagged output projections between attention
heads so the PE stream never waits on the exp chain.
"""

import sys

sys.path.insert(0, "/opt/trn_rl_repo")

import numpy as np

import concourse.bacc as bacc
import concourse.mybir as mybir
import concourse.tile as tile
from concourse.bass import ds, ts
from concourse.bass_utils import run_bass_kernel_spmd

B, T, D, H, DH = 2, 2048, 1024, 16, 64
G = 4            # head groups (tensor-parallel)
HPC = H // G     # heads per core
DG = D // G      # model dims per core (256)
P = 128
N_CORES = 8
NEG = -1.0e30
CUT = 24.0       # ALiBi cutoff in score units; dropped mass < T*e^(4-CUT)

F32 = mybir.dt.float32
F32R = mybir.dt.float32r
BF16 = mybir.dt.bfloat16
ADD = mybir.AluOpType.add
MULT = mybir.AluOpType.mult
EXP = mybir.ActivationFunctionType.Exp
COPYF = mybir.ActivationFunctionType.Copy

TRACE = False
LAST_RESULTS = None

_cache = {}

# windows for local head slot h: worst case over groups is group 0
# (slope = (h+1)/16), so W = CUT*16/(h+1) keys is safe for every core.
WIN = [CUT * 16.0 / (h + 1) for h in range(HPC)]


def _jc_min(h, qb):
    o = qb * 512
    return max(0, int(np.ceil((o - WIN[h] - 127) / 128.0)))


def _build(with_bias: bool):
    nc = bacc.Bacc("TRN2", target_bir_lowering=False, debug=False)

    x_d = nc.dram_tensor("xTb", [4, P, 8, 512], BF16, kind="ExternalInput").ap()
    wq_d = nc.dram_tensor("wqT", [P, 8, DG], BF16, kind="ExternalInput").ap()
    wk_d = nc.dram_tensor("wkT", [P, 8, DG], BF16, kind="ExternalInput").ap()
    wv_d = nc.dram_tensor("wvT", [P, 8, HPC, DH], BF16, kind="ExternalInput").ap()
    wo_d = nc.dram_tensor("woT", [P, 2, D], BF16, kind="ExternalInput").ap()
    qaug_d = nc.dram_tensor("qaug", [3, HPC, T], BF16, kind="ExternalInput").ap()
    kaug_d = nc.dram_tensor("kaug", [3, HPC, T], BF16, kind="ExternalInput").ap()
    ident_d = nc.dram_tensor("ident", [P, P], BF16, kind="ExternalInput").ap()
    maskst_d = nc.dram_tensor("maskst", [P, P], BF16, kind="ExternalInput").ap()
    if with_bias:
        bq_d = nc.dram_tensor("bq2", [P, 2], F32, kind="ExternalInput").ap()
        bk_d = nc.dram_tensor("bk2", [P, 2], F32, kind="ExternalInput").ap()
        bvo_d = nc.dram_tensor("bvo", [P, HPC, DH], F32, kind="ExternalInput").ap()
    out_d = nc.dram_tensor("outT", [D, T], BF16, kind="ExternalOutput").ap()

    with tile.TileContext(nc) as tc:
        with (
            tc.tile_pool(name="big", bufs=1) as big,
            tc.tile_pool(name="stage", bufs=3) as stage,
            tc.tile_pool(name="expp", bufs=5) as expp,
            tc.tile_pool(name="dnp", bufs=2) as dnp,
            tc.tile_pool(name="mm", bufs=2, space="PSUM") as mmp,
            tc.tile_pool(name="pss", bufs=4, space="PSUM") as pssp,
            tc.tile_pool(name="psy", bufs=2, space="PSUM") as psyp,
        ):
            # ---- persistent tiles
            wv = big.tile([P, 8, HPC, DH], BF16, tag="wv", name="wv")
            wq = big.tile([P, 8, DG], BF16, tag="wq", name="wq")
            wk = big.tile([P, 8, DG], BF16, tag="wk", name="wk")
            wo = big.tile([P, 2, D], BF16, tag="wo", name="wo")
            qa = big.tile([67, HPC, T], BF16, tag="qa", name="qa")
            ka = big.tile([67, HPC, T], BF16, tag="ka", name="ka")
            va = big.tile([P, 16, HPC, 65], BF16, tag="va", name="va")
            yt = [big.tile([P, T], BF16, tag=f"yt{m}", name=f"yt{m}") for m in range(2)]
            ident_sb = big.tile([P, P], BF16, tag="ident")
            maskst_sb = big.tile([P, P], BF16, tag="maskst")
            ones64 = big.tile([1, 64], F32, tag="ones64")
            xt = [big.tile([P, 8, 512], BF16, tag=f"xt{t}", name=f"xt{t}") for t in range(4)]
            if with_bias:
                bq2 = big.tile([P, 2], F32, tag="bq2")
                bk2 = big.tile([P, 2], F32, tag="bk2")
                bvo = big.tile([P, HPC, DH], F32, tag="bvo")

            # ---- input DMAs: few, large, ordered so the first t-block and
            # the weights land before the PE needs them.
            nc.sync.dma_start(out=ident_sb[:], in_=ident_d[:])
            nc.sync.dma_start(out=maskst_sb[:], in_=maskst_d[:])
            nc.sync.dma_start(out=xt[0][:, 0:4, :], in_=x_d[0][:, 0:4, :])
            nc.sync.dma_start(out=wk[:], in_=wk_d[:])
            nc.sync.dma_start(out=ka[64:67, :, :], in_=kaug_d[:])
            nc.sync.dma_start(out=qa[64:67, :, :], in_=qaug_d[:])
            nc.sync.dma_start(out=wo[:], in_=wo_d[:])
            for tq in range(1, 4):
                nc.sync.dma_start(out=xt[tq][:], in_=x_d[tq])

            nc.scalar.dma_start(out=wq[:], in_=wq_d[:])
            nc.scalar.dma_start(out=xt[0][:, 4:8, :], in_=x_d[0][:, 4:8, :])
            nc.scalar.dma_start(out=wv[:], in_=wv_d[:])
            if with_bias:
                nc.scalar.dma_start(out=bq2[:], in_=bq_d[:])
                nc.scalar.dma_start(out=bk2[:], in_=bk_d[:])
                nc.scalar.dma_start(out=bvo[:], in_=bvo_d[:])

            # preload the Exp table while waiting on weights
            nc.vector.memset(ones64[0:1, :], 1.0)
            dummy = dnp.tile([1, 512], F32, tag="dn", name="dummy")
            nc.scalar.activation(out=dummy[0:1, 0:64], in_=ones64[0:1, :], func=EXP)

            # ones columns for the in-matmul softmax denominator
            for ch in range(16):
                nc.gpsimd.memset(va[:, ch, :, 64:65], 1.0)

            # ---- PE warmup: ramp the p-state while the x/weight DMAs land
            for i in range(24):
                wps = pssp.tile([P, 512], F32, tag="pss", name=f"warm{i}")
                nc.tensor.matmul(out=wps[:, 0:P], lhsT=ident_sb[:], rhs=maskst_sb[:], start=True, stop=True)

            def emit_vproj(tq, ch):
                lc = (ch % 4) * P
                pv = mmp.tile([P, HPC, DH], F32, tag="mm", name=f"pv{ch}")
                for kc in range(8):
                    nc.tensor.matmul(
                        out=pv[:, :, :],
                        lhsT=xt[tq][:, kc, lc : lc + P],
                        rhs=wv[:, kc, :, :],
                        start=(kc == 0),
                        stop=(kc == 7),
                    )
                if with_bias:
                    nc.vector.tensor_tensor(
                        out=va[:, ch, :, 0:64], in0=pv[:, :, :], in1=bvo[:, :, :], op=ADD
                    )
                else:
                    nc.scalar.activation(out=va[:, ch, :, 0:64], in_=pv[:, :, :], func=COPYF)

            def emit_qkproj(tq, piece):
                wt, dst = [(wq, qa), (wk, ka)][piece // 2]
                mc = piece % 2
                pq = mmp.tile([P, 512], F32, tag="mm", name=f"pq{tq}_{piece}")
                for kc in range(8):
                    nc.tensor.matmul(
                        out=pq[:],
                        lhsT=wt[:, kc, ts(mc, P)],
                        rhs=xt[tq][:, kc, :],
                        start=(kc == 0),
                        stop=(kc == 7),
                    )
                bt = (bq2 if piece // 2 == 0 else bk2) if with_bias else None
                for half in range(2):
                    h = 2 * mc + half
                    src = pq[half * 64 : half * 64 + 64, :]
                    if with_bias:
                        nc.vector.tensor_scalar(
                            out=dst[0:64, h, ts(tq, 512)],
                            in0=src,
                            scalar1=bt[half * 64 : half * 64 + 64, mc : mc + 1],
                            scalar2=None,
                            op0=ADD,
                        )
                    else:
                        nc.vector.tensor_copy(out=dst[0:64, h, ts(tq, 512)], in_=src)

            pend_a = [None]  # (qb, h, py) awaiting recip+broadcast
            pend_b = [None]  # (qb, h, py, pbs) awaiting the normalize multiply

            def flush_recip():
                pqb, ph, ppy = pend_a[0]
                dnin = dnp.tile([1, 512], F32, tag="dnin", name=f"dnin{pqb}_{ph}")
                nc.vector.tensor_copy(out=dnin[0:1, :], in_=ppy[64:65, :])
                dn2 = dnp.tile([1, 512], F32, tag="dn", name=f"dn{pqb}_{ph}")
                nc.vector.reciprocal_approx_fast(out=dn2[0:1, :], in_=dnin[0:1, :])
                pbs = stage.tile([64, 512], F32, tag="pbs", name=f"pbs{pqb}_{ph}")
                nc.gpsimd.partition_broadcast(pbs[:, :], dn2[0:1, :])
                pend_a[0] = None
                pend_b[0] = (pqb, ph, ppy, pbs)

            def flush_mult():
                pqb, ph, ppy, pbs = pend_b[0]
                nc.vector.tensor_tensor(
                    out=yt[ph // 2][(ph % 2) * 64 : (ph % 2) * 64 + 64, ts(pqb, 512)],
                    in0=ppy[0:64, :],
                    in1=pbs[:, :],
                    op=MULT,
                )
                pend_b[0] = None

            def flush_all():
                if pend_a[0] is not None:
                    flush_recip()
                if pend_b[0] is not None:
                    flush_mult()

            def emit_head(qb, h, fillers=()):
                fillers = list(fillers)
                o = qb * 512
                jcs = list(range(_jc_min(h, qb), 4 * qb + 4))
                py = psyp.tile([65, 512], F32, tag="psy", name=f"py{qb}_{h}")
                q = []
                first = [True]

                def av(item, stop):
                    jc, lo, ex = item
                    nc.tensor.matmul(
                        out=py[:, lo:512],
                        lhsT=va[:, jc, h, :],
                        rhs=ex[:, lo:512],
                        start=first[0],
                        stop=stop,
                    )
                    first[0] = False

                for idx, jc in enumerate(jcs):
                    r = jc * P - o
                    lo = max(r, 0)
                    ps = pssp.tile([P, 512], F32, tag="pss", name=f"ps{qb}_{h}_{jc}")
                    if r < 0:
                        nc.tensor.matmul(
                            out=ps[:],
                            lhsT=ka[:, h, ts(jc, P)],
                            rhs=qa[:, h, ds(o, 512)],
                            start=True,
                            stop=True,
                        )
                    else:
                        nc.tensor.matmul(
                            out=ps[:, lo:512],
                            lhsT=ka[:, h, ts(jc, P)],
                            rhs=qa[:, h, ds(o + lo, 512 - lo)],
                            start=True,
                            stop=False,
                        )
                        # causal stair: ps[:, lo:lo+128] += I.T @ maskst
                        nc.tensor.matmul(
                            out=ps[:, lo : lo + P],
                            lhsT=ident_sb[:],
                            rhs=maskst_sb[:],
                            start=False,
                            stop=True,
                        )
                    ex = expp.tile([P, 512], BF16, tag="ex", name=f"ex{qb}_{h}_{jc}")
                    nc.scalar.activation(out=ex[:, lo:512], in_=ps[:, lo:512], func=EXP)
                    q.append((jc, lo, ex))
                    if idx == 1 and pend_a[0] is not None:
                        flush_recip()
                    if idx == 3 and pend_b[0] is not None:
                        flush_mult()
                    if idx in (4, 6) and fillers:
                        fillers.pop(0)()
                    if len(q) > 3:
                        av(q.pop(0), False)
                while q:
                    av(q.pop(0), len(q) == 1)
                for f in fillers:
                    f()
                flush_all()  # safety for short heads
                pend_a[0] = (qb, h, py)

            def emit_outproj(qb, ec, act_ok):
                po = mmp.tile([P, 512], F32, tag="mm", name=f"po{qb}_{ec}")
                for k2 in range(2):
                    nc.tensor.matmul(
                        out=po[:],
                        lhsT=wo[:, k2, ts(ec, P)],
                        rhs=yt[k2][:, ts(qb, 512)],
                        start=(k2 == 0),
                        stop=(k2 == 1),
                    )
                ob = stage.tile([P, 512], BF16, tag="ob", name=f"ob{qb}_{ec}")
                if act_ok and ec % 2 == 0:
                    nc.scalar.activation(out=ob[:], in_=po[:], func=COPYF)
                else:
                    nc.vector.tensor_copy(out=ob[:], in_=po[:])
                eng = nc.sync if ec % 2 == 0 else nc.gpsimd
                eng.dma_start(out=out_d[ts(ec, P), ts(qb, 512)], in_=ob[:])

            # ---- schedule: for tq0 emit the head-0 q/k pieces first so
            # attention(0) can start while v streams in behind.
            emit_qkproj(0, 0)
            emit_qkproj(0, 2)
            for j in range(4):
                emit_vproj(0, j)
            emit_qkproj(0, 1)
            emit_qkproj(0, 3)

            def op_filler(qb, ec, act_ok):
                return lambda: emit_outproj(qb, ec, act_ok)

            for qb in range(4):
                for h in range(HPC):
                    fillers = []
                    if qb > 0:
                        act_ok = qb < 3
                        fillers = [op_filler(qb - 1, ec, act_ok) for ec in (2 * h, 2 * h + 1)]
                    emit_head(qb, h, fillers)
                    if qb < 3:
                        emit_vproj(qb + 1, (qb + 1) * 4 + h)
                        emit_qkproj(qb + 1, h)
            # keep the PE p-state hot through the final normalize chain
            for i in range(14):
                wps = pssp.tile([P, 512], F32, tag="pss", name=f"tw{i}")
                nc.tensor.matmul(out=wps[:, 0:P], lhsT=ident_sb[:], rhs=maskst_sb[:], start=True, stop=True)
            flush_all()
            for ec in range(8):
                emit_outproj(3, ec, act_ok=True)

    nc.compile()
    return nc


def _get_nc(with_bias: bool):
    if with_bias not in _cache:
        _cache[with_bias] = _build(with_bias)
    return _cache[with_bias]


def kernel(x, freqs_cis, Wq, bq, Wkv, bkv, Wo, bo, **_unused):
    import ml_dtypes

    bf = ml_dtypes.bfloat16
    x = np.asarray(x, np.float32)
    Wq = np.asarray(Wq, np.float32)
    bq = np.asarray(bq, np.float32)
    Wkv = np.asarray(Wkv, np.float32)
    bkv = np.asarray(bkv, np.float32)
    Wo = np.asarray(Wo, np.float32)
    bo = np.asarray(bo, np.float32)

    with_bias = bool(np.any(bq) or np.any(bkv))
    nc = _get_nc(with_bias)

    scale = 1.0 / np.sqrt(DH)
    iota = np.arange(T, dtype=np.float32)
    j_hi = np.floor(iota / 256.0) * 256.0        # bf16-exact
    j_lo = iota - j_hi                           # 0..255, bf16-exact
    kaug1 = np.stack([j_hi, j_lo, np.ones(T, np.float32)])          # [3,T]
    kaug = np.broadcast_to(kaug1[:, None, :], (3, HPC, T)).astype(bf)

    mm = np.arange(P, dtype=np.float32)
    maskst = np.where(mm[None, :] < mm[:, None], NEG, 0.0).astype(bf)
    ident = np.eye(P, dtype=np.float32).astype(bf)

    # x pre-tiled to the SBUF layout: [tq, partition, kc, t] so each t-block
    # is one contiguous DMA.
    xTb = [
        np.ascontiguousarray(
            x[b].T.reshape(8, P, 4, 512).transpose(2, 1, 0, 3)
        ).astype(bf)
        for b in range(B)
    ]

    in_maps = []
    for c in range(N_CORES):
        b, g = divmod(c, G)
        rows = slice(g * DG, (g + 1) * DG)
        # [D, DG] -> [P, 8, DG] (partition-major with kc on the free dim)
        wqT = np.ascontiguousarray(
            (Wq[rows] * scale).T.reshape(8, P, DG).transpose(1, 0, 2)
        ).astype(bf)
        wkT = np.ascontiguousarray(
            Wkv[0:D][rows].T.reshape(8, P, DG).transpose(1, 0, 2)
        ).astype(bf)
        wvT = np.ascontiguousarray(
            Wkv[D : 2 * D][rows].T.reshape(8, P, HPC, DH).transpose(1, 0, 2, 3)
        ).astype(bf)
        woT = np.ascontiguousarray(
            Wo[:, rows].T.reshape(2, P, D).transpose(1, 0, 2)
        ).astype(bf)
        qaug = np.zeros((3, HPC, T), np.float32)
        for h in range(HPC):
            slope = (g * HPC + h + 1) / H
            qaug[0, h, :] = slope
            qaug[1, h, :] = slope
            qaug[2, h, :] = -slope * iota
        m = {
            "xTb": xTb[b],
            "wqT": wqT,
            "wkT": wkT,
            "wvT": wvT,
            "woT": woT,
            "qaug": qaug.astype(bf),
            "kaug": kaug,
            "ident": ident,
            "maskst": maskst,
        }
        if with_bias:
            m["bq2"] = np.ascontiguousarray((bq[rows] * scale).reshape(2, P).T)
            m["bk2"] = np.ascontiguousarray(bkv[0:D][rows].reshape(2, P).T)
            m["bvo"] = np.broadcast_to(
                bkv[D : 2 * D][rows].reshape(HPC, DH)[None], (P, HPC, DH)
            ).astype(np.float32).copy()
        in_maps.append(m)

    res = run_bass_kernel_spmd(nc, in_maps, list(range(N_CORES)), trace=TRACE)
    global LAST_RESULTS
    LAST_RESULTS = res

    out = np.empty((B, T, D), np.float32)
    for b in range(B):
        acc = res.results[b * G]["outT"].astype(np.float32)
        for g in range(1, G):
            acc += res.results[b * G + g]["outT"].astype(np.float32)
        out[b] = acc.T + bo[None, :]
    return out
